# revision 4
# baseline (speedup 1.0000x reference)
"""Trainium2 Bass kernel for nn_Denoiser (dense MLP 2->16->16x5->2, N=4194304).

Strategy (pure data parallel over 8 NeuronCores; evacuation-optimized v2):
  - Shard the batch over 8 cores (524288 points each); weights replicated.
  - Block-diagonal weights stack 8 batch groups on SBUF partitions so each
    [128,512] matmul column carries 8 points (f32r, fp32 PSUM).
  - The bottleneck on TRN2 is PSUM evacuation: PSUM is fp32-only and only
    DVE/ACT can read it at 1 elem/lane/cycle, so ~98 fp32 values per point
    must stream through those two engines. v2 minimizes per-op overhead by
    evacuating FOUR banks at a time: a "conveyor" of layer-steps writes
    4-bank PSUM quads ([128,2048] tiles, ping-ponged 2-deep), and each quad
    is evacuated by ONE fused relu op (FD=2048) on DVE or ACT, chosen by a
    deficit scheduler (DVE ~2.26us/op, ACT ~2.0us/op).
  - Per 8-chunk period (chunk = 4096 pts): two interleaved 4-chunk
    super-lines (A/B) give every evac->consumer dependency a lag of >= 2
    quads, so PE matmuls always overlap engine evacuation with only 8 banks.
    Period = 13 quads: L0A L0B PK(prev) L1A L1B ... L5A L5B, where PK packs
    the 16->2 output layer for 8 chunks as 8 PSUM-accumulated matmuls into
    one bank (block-shifted w6 copies), evacuated by a single [128,512] copy
    and DMA'd without padding.
  - L0 weights are zero-padded to K=128 so every matmul runs in the same
    (128,128) PE tiling mode (no TensorE mode-switch drains).
  - The host pre-permutes x into [16, S*C] (partition = 2*group+feature) so
    x DMAs are contiguous [16,2048] slices; output [128, 16*512] is decoded
    on the host exactly as in the data layout of the pack weights.
"""

import numpy as np

N = 4194304
N_CORES = 8
N_SHARD = N // N_CORES  # 524288
G = 8            # batch groups stacked along partitions
C = 512          # batch columns per chunk (1 PSUM bank per matmul)
S = N_SHARD // (G * C)   # 128 chunks per core
PER = 8          # chunks per period (pack group)
N_PER = S // PER  # 16 periods
HALF = PER // 2  # 4 chunks per super-line
N_NODE = 16
N_INT = 5

_CACHE = {}

TRACE = False
LAST_RESULT = None

# measured-ish per-op engine costs (ns) for the deficit scheduler
_DVE_RELU_2048 = 2360
_ACT_RELU_2048 = 2100
_DVE_COPY_512 = 760
_ACT_COPY_512 = 680


def _build_bass():
    from contextlib import ExitStack

    import concourse.mybir as mybir
    import concourse.tile as tile
    from concourse import bacc

    f32 = mybir.dt.float32
    f32r = mybir.dt.float32r
    nc = bacc.Bacc("TRN2", target_bir_lowering=False, num_devices=N_CORES)

    # xd[2g+f, s*C + c] = x[s*G*C + g*C + c, f]   (host pre-permuted)
    xd = nc.dram_tensor("xd", [16, S * C], f32r, kind="ExternalInput")
    w0 = nc.dram_tensor("w0", [128, 128], f32r, kind="ExternalInput")
    wm = nc.dram_tensor("wm", [N_INT, 128, 128], f32r, kind="ExternalInput")
    w6 = nc.dram_tensor("w6", [8, 128, 128], f32r, kind="ExternalInput")
    # yd[16*j+2g+f, m*C + c] = y[(8m+j)*G*C + g*C + c, f]
    yd = nc.dram_tensor("yd", [128, N_PER * C], f32, kind="ExternalOutput")

    QW = 4 * C  # quad width (2048)

    with tile.TileContext(nc) as tc, ExitStack() as ctx:
        wpool = ctx.enter_context(tc.tile_pool(name="weights", bufs=1))
        xpool = ctx.enter_context(tc.tile_pool(name="x", bufs=1))
        hpool = ctx.enter_context(tc.tile_pool(name="h", bufs=8))
        opool = ctx.enter_context(tc.tile_pool(name="o", bufs=2))
        qpool = ctx.enter_context(tc.tile_pool(name="ps", bufs=2, space="PSUM"))

        # ---- weights ----
        w0_t = wpool.tile([128, 128], f32r, tag="w0", name="w0_t")
        nc.sync.dma_start(out=w0_t, in_=w0[:, :])
        wm_t = []
        for l in range(N_INT):
            t = wpool.tile([128, 128], f32r, tag=f"wm{l}", name=f"wm{l}_t")
            (nc.sync if l % 2 == 0 else nc.scalar).dma_start(out=t, in_=wm[l, :, :])
            wm_t.append(t)
        w6_t = []
        for j in range(8):
            t = wpool.tile([128, 128], f32r, tag=f"w6{j}", name=f"w6{j}_t")
            (nc.scalar if j % 2 == 0 else nc.sync).dma_start(out=t, in_=w6[j, :, :])
            w6_t.append(t)
        lhsT = [w0_t] + wm_t  # stationary for layers 0..5

        # ---- x tiles: manual rotation so the one-time memset sticks ----
        N_XT = 4
        xts = []
        for i in range(N_XT):
            t = xpool.tile([128, QW], f32r, tag=f"xt{i}", name=f"xt{i}")
            nc.gpsimd.memset(t.bitcast(mybir.dt.uint32), 0)
            xts.append(t)

        def dma_x(idx, first_chunk):
            nc.gpsimd.dma_start(
                out=xts[idx % N_XT][0:16, :],
                in_=xd[:, first_chunk * C : (first_chunk + HALF) * C],
            )

        # engine deficit scheduler
        sched = {"dve": 0.0, "act": 0.0}

        def evac(dst, src, relu, cost_dve, cost_act):
            if sched["dve"] + cost_dve <= sched["act"] + cost_act:
                sched["dve"] += cost_dve
                if relu:
                    nc.vector.tensor_scalar_max(dst, src, 0.0)
                else:
                    nc.vector.tensor_copy(dst, src)
            else:
                sched["act"] += cost_act
                if relu:
                    nc.scalar.activation(dst, src, mybir.ActivationFunctionType.Relu)
                else:
                    nc.scalar.copy(dst, src)

        # prefetch first two periods of x
        for m in (0, 1):
            dma_x(2 * m, PER * m)
            dma_x(2 * m + 1, PER * m + HALF)

        h_tiles = {}  # (layer, half, period) -> SBUF tile [128, QW]
        nq = 0  # quad counter (for names)

        def relu_quad(l, half, m):
            """Layer l (0..5) of super-line half (0/1) of period m."""
            nonlocal nq
            qt = qpool.tile([128, QW], f32, tag="q", name=f"q{nq}")
            if l == 0:
                src = xts[(2 * m + half) % N_XT]
            else:
                src = h_tiles[(l - 1, half, m)]
            for k in range(4):
                nc.tensor.matmul(
                    qt[:, k * C : (k + 1) * C],
                    lhsT[l],
                    src[:, k * C : (k + 1) * C],
                    start=True,
                    stop=True,
                    skip_group_check=True,
                )
            ht = hpool.tile([128, QW], f32r, tag="h", name=f"h{nq}")
            evac(ht, qt, True, _DVE_RELU_2048, _ACT_RELU_2048)
            h_tiles[(l, half, m)] = ht
            if l > 0:
                del h_tiles[(l - 1, half, m)]
            nq += 1

        def pack_quad(m):
            """Output layer for all 8 chunks of period m -> one PSUM bank."""
            nonlocal nq
            qt = qpool.tile([128, QW], f32, tag="q", name=f"q{nq}")
            for j in range(8):
                src = h_tiles[(5, j // HALF, m)]
                nc.tensor.matmul(
                    qt[:, 0:C],
                    w6_t[j],
                    src[:, (j % HALF) * C : (j % HALF + 1) * C],
                    start=(j == 0),
                    stop=(j == 7),
                    skip_group_check=True,
                )
            del h_tiles[(5, 0, m)]
            del h_tiles[(5, 1, m)]
            ot = opool.tile([128, C], f32, tag="o", name=f"o{nq}")
            evac(ot, qt[:, 0:C], False, _DVE_COPY_512, _ACT_COPY_512)
            nc.sync.dma_start(out=yd[:, m * C : (m + 1) * C], in_=ot)
            nq += 1

        for m in range(N_PER):
            relu_quad(0, 0, m)
            relu_quad(0, 1, m)
            if m + 2 < N_PER:
                dma_x(2 * (m + 2), PER * (m + 2))
                dma_x(2 * (m + 2) + 1, PER * (m + 2) + HALF)
            if m > 0:
                pack_quad(m - 1)
            for l in range(1, 6):
                relu_quad(l, 0, m)
                relu_quad(l, 1, m)
        pack_quad(N_PER - 1)
    nc.compile()
    return nc


def _prep_weights(w_in, w_mid, w_out):
    """Block-diagonal stationary operands (lhsT = W.T blocks) for 8 groups."""
    w0 = np.zeros((128, 128), dtype=np.float32)
    for g in range(G):
        w0[2 * g : 2 * g + 2, 16 * g : 16 * g + 16] = w_in.T  # [2,16]
    wm = np.zeros((N_INT, 128, 128), dtype=np.float32)
    for l in range(N_INT):
        for g in range(G):
            wm[l, 16 * g : 16 * g + 16, 16 * g : 16 * g + 16] = w_mid[l].T
    w6 = np.zeros((8, 128, 128), dtype=np.float32)
    for j in range(8):
        for g in range(G):
            w6[j, 16 * g : 16 * g + 16, 16 * j + 2 * g : 16 * j + 2 * g + 2] = (
                w_out.T
            )  # [16,2]
    return w0, wm, w6


def _shard_x(shard):
    """[N_SHARD, 2] -> [16, S*C] with row 2g+f, col s*C+c."""
    v = shard.reshape(S, G, C, 2)           # [s, g, c, f]
    v = v.transpose(1, 3, 0, 2)             # [g, f, s, c]
    return np.ascontiguousarray(v.reshape(16, S * C))


def _unshard_y(yd):
    """[128, N_PER*C] -> [N_SHARD, 2].  Row q = 16*j + 2*g + f."""
    v = yd.reshape(PER, G, 2, N_PER, C)               # [j, g, f, m, c]
    v = v.transpose(3, 0, 1, 4, 2)                    # [m, j, g, c, f]
    return v.reshape(N_SHARD, 2)


def kernel(x, w_in, w_mid, w_out):
    from concourse.bass_utils import run_bass_kernel_spmd

    x = np.ascontiguousarray(x, dtype=np.float32)
    w0, wm, w6 = _prep_weights(
        np.asarray(w_in, dtype=np.float32),
        np.asarray(w_mid, dtype=np.float32),
        np.asarray(w_out, dtype=np.float32),
    )

    if "nc" not in _CACHE:
        _CACHE["nc"] = _build_bass()
    nc = _CACHE["nc"]

    in_maps = []
    for c in range(N_CORES):
        shard = x[c * N_SHARD : (c + 1) * N_SHARD]
        in_maps.append({"xd": _shard_x(shard), "w0": w0, "wm": wm, "w6": w6})

    res = run_bass_kernel_spmd(
        nc, in_maps, core_ids=list(range(N_CORES)), trace=TRACE
    )
    global LAST_RESULT
    LAST_RESULT = res
    y = np.empty((N, 2), dtype=np.float32)
    for c in range(N_CORES):
        y[c * N_SHARD : (c + 1) * N_SHARD] = _unshard_y(res.results[c]["yd"])
    return y
